# revision 9
# baseline (speedup 1.0000x reference)
"""2-layer LSTM (B=256, T=512, F=49, H=512) on 8 trn2 NeuronCores.

Data-parallel over batch: core j owns batch rows [j*32, (j+1)*32), holds the
full (replicated) LSTM weights, and runs the whole recurrence locally.

Folded layout (this version): gates PSUM is [128, 512] where partition
p = 32*j + b (j = hidden chunk of 128 units, b = batch row) and free dim is
[i|f|o|g] x 128 columns.  Each GEMM k-slot issues 4 matmuls (one per hidden
chunk j) whose outputs land at partition offsets 32j -> tile_position (0,32j)
col-tiling runs them concurrently on the PE array, so a k-slot costs ~one
N=512 stream instead of four.  All elementwise tiles are [128, 128] (full
partition utilization on ACT/DVE).  h chunks live at partition offsets 32j,
so the four h->hT PE transposes are row-tiled (concurrent).  The output
projection rides the PE as N=1 matvec columns accumulated into a dedicated
PSUM bank across all T.
"""

import sys
import types

import numpy as np

# ---------------------------------------------------------------------------
# axon NTFF profile hook (degrades silently if unavailable)
# ---------------------------------------------------------------------------


def _install_axon_hook():
    if "antenv.axon_hooks" in sys.modules:
        return
    mod = types.ModuleType("antenv.axon_hooks")
    mod._hook = None
    mod.set_axon_ntff_profile_hook = lambda h: setattr(mod, "_hook", h)
    mod.get_axon_ntff_profile_hook = lambda: mod._hook
    sys.modules["antenv.axon_hooks"] = mod
    try:
        import antenv

        antenv.axon_hooks = mod
        from trn_agent_boot.trn_boot import _ntff_profile_via_ctypes

        hook = _ntff_profile_via_ctypes("/opt/axon/libaxon_pjrt.so")
        if hook is not None:
            mod.set_axon_ntff_profile_hook(hook)
    except Exception:
        pass


_install_axon_hook()

import concourse.bacc as bacc
import concourse.mybir as mybir
from concourse.bass_utils import run_bass_kernel_spmd

DT = mybir.dt.float32
BF = mybir.dt.bfloat16
AF = mybir.ActivationFunctionType

B, T_FULL, F, H = 256, 512, 49, 512
BL = 32  # batch rows per core
FCH = 50  # features + ones row
# gate order in the packed free dim: i, f, o, g (sigmoid gates contiguous)
GSEL = [0, 1, 3, 2]  # packed slot -> pytorch gate index (i,f,g,o order)


def _matmul_noldw(pe, out, lhsT, rhs, start, stop, tile_position, tile_size=(128, 32)):
    """Matmul that does not emit its own LDWEIGHTS (array already loaded)."""
    ifmap_ap = pe.lower_ap(rhs.opt({0}), opt=False)
    weights_ap = pe.lower_ap(lhsT.opt({0}), opt=False, for_matmul_weights=True)
    out_ap = pe.lower_ap(out)
    return pe.add_instruction(
        mybir.InstMatmult(
            name=pe.bass.get_next_instruction_name(),
            replication_resolution=0,
            replication_shift_amnt=0,
            replication_num_rows=0,
            start_tensor_calc=start,
            stop_tensor_calc=stop,
            ins=[ifmap_ap, weights_ap],
            outs=[out_ap],
            perf_mode=None,
            is_transpose=None,
            ifmap_quant_offset=None,
            weights_quant_offset=None,
            bass_skip_group_check=True,
            tile_position=tile_position,
            tile_size=tile_size,
            ldweights=False,
        )
    )


def build(T=T_FULL):
    TCH = min(64, T)
    n_chunks = (T + TCH - 1) // TCH
    nc = bacc.Bacc("TRN2", target_bir_lowering=False)

    xw_d = nc.dram_tensor("xw", [FCH, T, BL], BF, kind="ExternalInput")
    wih1_d = nc.dram_tensor("wih1", [FCH, 2048], BF, kind="ExternalInput")
    whh1_d = nc.dram_tensor("whh1", [128, 8192], BF, kind="ExternalInput")
    wih2_d = nc.dram_tensor("wih2", [128, 8192], BF, kind="ExternalInput")
    whh2_d = nc.dram_tensor("whh2", [128, 8192], BF, kind="ExternalInput")
    b2_d = nc.dram_tensor("b2", [1, 2048], BF, kind="ExternalInput")
    woutT_d = nc.dram_tensor("woutT", [128, 4], BF, kind="ExternalInput")
    ident_d = nc.dram_tensor("ident", [128, 128], BF, kind="ExternalInput")
    out_d = nc.dram_tensor("out", [BL, T], DT, kind="ExternalOutput")

    xw_sb = nc.alloc_sbuf_tensor("xw_sb", [FCH, 2, TCH, BL], BF)
    wih1 = nc.alloc_sbuf_tensor("wih1_sb", [FCH, 2048], BF)
    whh1 = nc.alloc_sbuf_tensor("whh1_sb", [128, 8192], BF)
    wih2 = nc.alloc_sbuf_tensor("wih2_sb", [128, 8192], BF)
    whh2 = nc.alloc_sbuf_tensor("whh2_sb", [128, 8192], BF)
    b2 = nc.alloc_sbuf_tensor("b2_sb", [1, 2048], BF)
    woutT = nc.alloc_sbuf_tensor("woutT_sb", [128, 4], BF)
    ident = nc.alloc_sbuf_tensor("ident_sb", [128, 128], BF)
    ones = nc.alloc_sbuf_tensor("ones_sb", [1, BL], BF)

    def par(name, w, dt):
        return [nc.alloc_sbuf_tensor(f"{name}_{p}", [128, w], dt) for p in range(2)]

    sA1, sA2 = par("sA1", 512, DT), par("sA2", 512, DT)
    thc1, thc2 = par("thc1", 128, DT), par("thc2", 128, DT)
    c1, c2 = par("c1", 128, DT), par("c2", 128, DT)
    h1L, h2L = par("h1L", 128, BF), par("h2L", 128, BF)
    hT1 = nc.alloc_sbuf_tensor("hT1", [128, 256], BF)
    hT2 = nc.alloc_sbuf_tensor("hT2", [128, 256], BF)
    tm1 = nc.alloc_sbuf_tensor("tm1", [128, 128], DT)
    ta1 = nc.alloc_sbuf_tensor("ta1", [128, 128], DT)
    tm2 = nc.alloc_sbuf_tensor("tm2", [128, 128], DT)
    ta2 = nc.alloc_sbuf_tensor("ta2", [128, 128], DT)
    out_sb = nc.alloc_sbuf_tensor("out_sb", [BL, T], DT)

    ps1 = [nc.alloc_psum_tensor(f"ps1_{p}", [128, 512], DT) for p in range(2)]
    ps2 = [nc.alloc_psum_tensor(f"ps2_{p}", [128, 512], DT) for p in range(2)]
    ptr1 = nc.alloc_psum_tensor("ptr1", [128, 2, 128], BF)
    ptr2 = nc.alloc_psum_tensor("ptr2", [128, 2, 128], BF)
    pso = nc.alloc_psum_tensor("pso", [BL, T], DT)

    S = lambda n: nc.alloc_semaphore(n)
    sem_w, sem_x, sem_ones = S("sem_w"), S("sem_x"), S("sem_ones")
    pe1, pe2 = S("pe1"), S("pe2")
    a1, a2 = S("a1"), S("a2")
    at1, at2 = S("at1"), S("at2")
    sc1, sc2 = S("sc1"), S("sc2")
    he1, he2 = S("he1"), S("he2")
    tr1, tr2 = S("tr1"), S("tr2")
    ho1, ho2 = S("ho1"), S("ho2")
    prj_pe, prj = S("prj_pe"), S("prj")

    with nc.Block() as block:

        @block.sync
        def _(sync):
            for dst, src in [
                (wih1, wih1_d), (whh1, whh1_d), (wih2, wih2_d), (whh2, whh2_d),
                (b2, b2_d), (woutT, woutT_d), (ident, ident_d),
            ]:
                sync.dma_start(out=dst[:], in_=src[:]).then_inc(sem_w, 16)
            for c in range(n_chunks):
                if c >= 2:
                    sync.wait_ge(pe1, TCH * (c - 1))
                inst = sync.dma_start(
                    out=xw_sb[:, c % 2], in_=xw_d[:, c * TCH : (c + 1) * TCH, :]
                )
                if c >= 1:
                    inst._wait_ge(sem_x, 16 * c)
                inst.then_inc(sem_x, 16)
            sync.wait_ge(prj, 1)
            sync.dma_start(out=out_d[:], in_=out_sb[:]).then_inc(sem_x, 16)
            sync.wait_ge(sem_x, 16 * (n_chunks + 1))

        @block.tensor
        def _(pe):
            pe.wait_ge(sem_w, 7 * 16)
            pe.wait_ge(sem_ones, 1)

            for t in range(T):
                pi = t % 2
                if t % TCH == 0:
                    pe.wait_ge(sem_x, 16 * (t // TCH + 1))
                xt = xw_sb[:, (t // TCH) % 2, t % TCH, :]

                # ---- L1 gemm -> ps1[pi] (x slot + 4 hh1 k-slots, col-tiled by j)
                if t >= 2:
                    pe.wait_ge(a1, t - 1)  # ps1[pi] free
                if t >= 1:
                    pe.wait_ge(ho1, t)     # hT1 = h1[t-1]
                for j in range(4):
                    mm = pe.matmul(
                        ps1[pi][32 * j : 32 * j + 32, :],
                        xt, wih1[:, 512 * j : 512 * j + 512],
                        start=True, stop=(t == 0),
                        tile_position=(0, 32 * j),
                    )
                if t >= 1:
                    for r in range(4):
                        pe.ldweights(hT1[:, 32 * r : 32 * r + 128])
                        for j in range(4):
                            k = (j + r) % 4
                            mm = _matmul_noldw(
                                pe,
                                ps1[pi][32 * j : 32 * j + 32, :],
                                hT1[:, 32 * r + 32 * j : 32 * r + 32 * j + 32],
                                whh1[:, k * 2048 + 512 * j : k * 2048 + 512 * j + 512],
                                start=False, stop=(r == 3),
                                tile_position=(0, 32 * j),
                            )
                mm.then_inc(pe1, 1)

                # ---- transpose h2[t-1] -> ptr2[pi] (row-tiled, concurrent)
                if t >= 1:
                    pe.wait_ge(he2, t)
                    if t >= 3:
                        pe.wait_ge(ho2, t - 2)  # ptr2[pi] WAR
                    pe.transpose(
                        ptr2[:, pi, :], h2L[(t - 1) % 2][:], ident[:]
                    ).then_inc(tr2, 1)

                # ---- L2 bias + hh2 + outproj(t-1) -> ps2[pi], pso[:, t-1]
                if t >= 2:
                    pe.wait_ge(a2, t - 1)  # ps2[pi] free
                for j in range(4):
                    pe.matmul(
                        ps2[pi][32 * j : 32 * j + 32, :],
                        ones[:], b2[:, 512 * j : 512 * j + 512],
                        start=True, stop=False,
                        tile_position=(0, 32 * j),
                    )
                if t >= 1:
                    pe.wait_ge(ho2, t)     # hT2 = h2[t-1]
                    for r in range(4):
                        pe.ldweights(hT2[:, 32 * r : 32 * r + 128])
                        for j in range(4):
                            k = (j + r) % 4
                            _matmul_noldw(
                                pe,
                                ps2[pi][32 * j : 32 * j + 32, :],
                                hT2[:, 32 * r + 32 * j : 32 * r + 32 * j + 32],
                                whh2[:, k * 2048 + 512 * j : k * 2048 + 512 * j + 512],
                                start=False, stop=False,
                                tile_position=(0, 32 * j),
                            )
                        _matmul_noldw(
                            pe,
                            pso[:, t - 1 : t],
                            hT2[:, 32 * r : 32 * r + 32],
                            woutT[:, r : r + 1],
                            start=(r == 0), stop=(r == 3),
                            tile_position=(0, 0),
                        )

                # ---- transpose h1[t] -> ptr1[pi]
                pe.wait_ge(he1, t + 1)
                if t >= 2:
                    pe.wait_ge(ho1, t - 1)  # ptr1[pi] WAR
                pe.transpose(ptr1[:, pi, :], h1L[pi][:], ident[:]).then_inc(tr1, 1)

                # ---- ih2 -> ps2[pi] (stop)
                pe.wait_ge(ho1, t + 1)     # hT1 = h1[t]
                for r in range(4):
                    pe.ldweights(hT1[:, 32 * r : 32 * r + 128])
                    for j in range(4):
                        k = (j + r) % 4
                        mm = _matmul_noldw(
                            pe,
                            ps2[pi][32 * j : 32 * j + 32, :],
                            hT1[:, 32 * r + 32 * j : 32 * r + 32 * j + 32],
                            wih2[:, k * 2048 + 512 * j : k * 2048 + 512 * j + 512],
                            start=False, stop=(r == 3),
                            tile_position=(0, 32 * j),
                        )
                mm.then_inc(pe2, 1)

            # ---- post-loop: transpose h2[T-1], outproj column T-1
            pe.wait_ge(he2, T)
            pe.wait_ge(ho2, T - 2)
            pe.transpose(
                ptr2[:, T % 2, :], h2L[(T - 1) % 2][:], ident[:]
            ).then_inc(tr2, 1)
            pe.wait_ge(ho2, T)
            for r in range(4):
                pe.ldweights(hT2[:, 32 * r : 32 * r + 128])
                mm = _matmul_noldw(
                    pe,
                    pso[:, T - 1 : T],
                    hT2[:, 32 * r : 32 * r + 32],
                    woutT[:, r : r + 1],
                    start=(r == 0), stop=(r == 3),
                    tile_position=(0, 0),
                )
            mm.then_inc(prj_pe, 1)

        @block.scalar
        def _(act):
            for t in range(T):
                pi = t % 2
                for (peS, sA, ps, thc, cc, ra, att, scs) in [
                    (pe1, sA1, ps1, thc1, c1, a1, at1, sc1),
                    (pe2, sA2, ps2, thc2, c2, a2, at2, sc2),
                ]:
                    act.wait_ge(peS, t + 1)
                    act.activation(sA[pi][:, 0:384], ps[pi][:, 0:384], AF.Sigmoid)
                    act.activation(
                        sA[pi][:, 384:512], ps[pi][:, 384:512], AF.Tanh
                    ).then_inc(ra, 1)
                    act.wait_ge(scs, t + 1)
                    act.activation(thc[pi][:], cc[pi][:], AF.Tanh).then_inc(att, 1)

        @block.vector
        def _(dve):
            for t in range(T):
                pi = t % 2
                po = 1 - pi
                # hT2 <- ptr2 (h2[t-1])
                if t >= 1:
                    dve.drain()  # cross-step same-engine RAW (c/h tiles)
                    dve.wait_ge(tr2, t)
                    dve.tensor_copy(hT2[:, 0:128], ptr2[:, pi, :])
                    dve.tensor_copy(hT2[:, 128:256], ptr2[:, pi, :]).then_inc(ho2, 1)
                for (sA, thc, cc, tm, ta, hh, ra, att, scs, hes, trs, hos, hT, ptr) in [
                    (sA1, thc1, c1, tm1, ta1, h1L, a1, at1, sc1, he1, tr1, ho1, hT1, ptr1),
                    (sA2, thc2, c2, tm2, ta2, h2L, a2, at2, sc2, he2, tr2, ho2, hT2, ptr2),
                ]:
                    dve.wait_ge(ra, t + 1)
                    if t == 0:
                        dve.tensor_mul(
                            cc[pi][:], sA[pi][:, 0:128], sA[pi][:, 384:512]
                        ).then_inc(scs, 1)
                    else:
                        dve.tensor_mul(tm[:], sA[pi][:, 0:128], sA[pi][:, 384:512])
                        dve.tensor_mul(ta[:], sA[pi][:, 128:256], cc[po][:])
                        dve.drain()
                        dve.tensor_add(cc[pi][:], ta[:], tm[:]).then_inc(scs, 1)
                    dve.wait_ge(att, t + 1)
                    dve.tensor_mul(hh[pi][:], sA[pi][:, 256:384], thc[pi][:]).then_inc(
                        hes, 1
                    )
                    if hT is hT1:
                        dve.wait_ge(trs, t + 1)
                        dve.tensor_copy(hT[:, 0:128], ptr[:, pi, :])
                        dve.tensor_copy(hT[:, 128:256], ptr[:, pi, :]).then_inc(hos, 1)
            # final hT2 copy + out copy
            dve.wait_ge(tr2, T)
            dve.tensor_copy(hT2[:, 0:128], ptr2[:, T % 2, :])
            dve.tensor_copy(hT2[:, 128:256], ptr2[:, T % 2, :]).then_inc(ho2, 1)
            dve.wait_ge(prj_pe, 1)
            dve.tensor_copy(out_sb[:], pso[:]).then_inc(prj, 1)

        @block.gpsimd
        def _(gp):
            gp.memset(ones[:], 1.0).then_inc(sem_ones, 1)

    nc.compile()
    return nc


def _pack_perm():
    """packed col p -> original W^T column (pytorch gate order i,f,g,o)."""
    p = np.arange(2048)
    j = p // 512
    gsel = (p % 512) // 128
    um = p % 128
    gate = np.array(GSEL)[gsel]
    return gate * 512 + j * 128 + um


_PERM = _pack_perm()


def prepack(inputs, core):
    x = np.asarray(inputs["x"], dtype=np.float32)
    T = x.shape[1]
    w_ih1 = np.asarray(inputs["w_ih1"], dtype=np.float32)
    w_hh1 = np.asarray(inputs["w_hh1"], dtype=np.float32)
    b1 = np.asarray(inputs["b_ih1"], dtype=np.float32) + np.asarray(
        inputs["b_hh1"], dtype=np.float32
    )
    w_ih2 = np.asarray(inputs["w_ih2"], dtype=np.float32)
    w_hh2 = np.asarray(inputs["w_hh2"], dtype=np.float32)
    b2v = np.asarray(inputs["b_ih2"], dtype=np.float32) + np.asarray(
        inputs["b_hh2"], dtype=np.float32
    )
    w_out = np.asarray(inputs["w_out"], dtype=np.float32)

    xs = x[core * BL : (core + 1) * BL]  # [32, T, 49]
    xw = np.empty((FCH, T, BL), np.float32)
    xw[:F] = np.transpose(xs, (2, 1, 0))
    xw[F] = 1.0

    wih1p = np.empty((FCH, 2048), np.float32)
    wih1p[:F] = w_ih1.T
    wih1p[F] = b1
    wih1p = wih1p[:, _PERM]

    def hh_pack(w):  # [2048, 512] -> [128, 4*2048] k-chunk-major, packed cols
        wt = w.T[:, _PERM]  # [512, 2048]
        out = np.empty((128, 4 * 2048), np.float32)
        for k in range(4):
            out[:, k * 2048 : (k + 1) * 2048] = wt[k * 128 : (k + 1) * 128, :]
        return out

    import ml_dtypes

    bf16 = ml_dtypes.bfloat16
    return {
        "xw": np.ascontiguousarray(xw).astype(bf16),
        "wih1": np.ascontiguousarray(wih1p).astype(bf16),
        "whh1": hh_pack(w_hh1).astype(bf16),
        "wih2": hh_pack(w_ih2).astype(bf16),
        "whh2": hh_pack(w_hh2).astype(bf16),
        "b2": np.ascontiguousarray(b2v[_PERM][None, :]).astype(bf16),
        "woutT": np.ascontiguousarray(w_out[0].reshape(4, 128).T).astype(bf16),
        "ident": np.eye(128, dtype=np.float32).astype(bf16),
    }


_NC_CACHE = {}


def _get_nc(T):
    if T not in _NC_CACHE:
        _NC_CACHE[T] = build(T)
    return _NC_CACHE[T]


def kernel(**inputs):
    """Full-input entry: shard over 8 cores, run, gather. Returns [B, T] fp32."""
    x = np.asarray(inputs["x"])
    T = x.shape[1]
    nc = _get_nc(T)
    in_maps = [prepack(inputs, j) for j in range(8)]
    res = run_bass_kernel_spmd(nc, in_maps, core_ids=list(range(8)))
    out = np.empty((B, T), np.float32)
    for j in range(8):
        out[j * BL : (j + 1) * BL] = res.results[j]["out"][:, :T]
    out += np.asarray(inputs["b_out"], dtype=np.float32)[0]
    return out
